# revision 4
# baseline (speedup 1.0000x reference)
"""GAT layer (PyG GATConv, concat=False, edge_dim=1) on 8 Trainium2 cores.

Sharding: core c = (batch b = c//2, dst-half = c%2). Graph structure is
replicated; each core computes h = x[b] @ W for all N nodes, then processes
only edges whose destination falls in its half of the node range.

Edges are sorted by destination on the host and grouped into 128-edge blocks
per 128-node destination tile, so the segment softmax and the scatter-add
become dense on-chip ops:
  - per-edge source rows [h | a_src] are fetched with dma_gather from an HBM
    table written in phase 1 (1280 B rows); a_dst rows (256 B) from a second
    table keyed by destination.
  - alpha = leakyrelu(a_src[src] + a_dst[dst] + attr*c); p = exp(alpha).
    No max-subtraction: |alpha| <= ~10 for this data, exp is safe in f32,
    and softmax is algebraically identical without it.
  - scatter: for each 128-edge block, an indicator matrix
    Ind[e, n] = (dst_local[e] == n) is built with iota + tensor_scalar
    is_equal, then PSUM[n, :] += Ind.T @ [p*h_src | p]  accumulates both the
    weighted-message numerator (256 cols) and the softmax denominator
    (4 cols) in one matmul group per destination tile.
  - epilogue per tile: divide by denominator, mean over 4 heads, + bias.

The program structure (block counts per tile) is shared across all 8 cores
(max over cores), so a single NEFF runs SPMD with per-core index/stream data.
"""

import numpy as np

# problem constants (hardcoded per contract -- no reading of spec/reference)
B, N, E, D, H, O = 4, 10000, 160000, 128, 4, 64
NEG_SLOPE = 0.2
P = 128
HO = H * O                       # 256
NHALF = N // 2                   # 5000
NT = (NHALF + P - 1) // P        # 40 destination tiles per half
NPAD = NT * P                    # 5120
NROWT = ((N + P - 1) // P) * P   # 10112  (table rows, node-tile padded)
N_NT = N // P + (1 if N % P else 0)  # 79 node tiles for h build (last partial)
ROW_A = 320                      # tableA row: h(256) | a_src(4) | pad -> 1280B
ROW_B = 64                       # tableB row: a_dst(4) | pad -> 256B
CHUNK = 8                        # blocks per gather call (1024 edges; HW limit)
NCORE = 8

_cache = {}


def _build_program(meta):
    import concourse.bacc as bacc
    import concourse.mybir as mybir
    from concourse.tile import TileContext
    from concourse.library_config import mlp

    f32 = mybir.dt.float32
    i16 = mybir.dt.int16
    i32 = mybir.dt.int32
    Alu = mybir.AluOpType
    Act = mybir.ActivationFunctionType

    nblk = meta["nblk"]
    blk_tile = meta["blk_tile"]        # [nblk] tile id of each block
    blk_first = meta["blk_first"]      # [nblk] bool: first block of its tile
    blk_last = meta["blk_last"]        # [nblk] bool: last block of its tile
    ne = nblk * P
    nch = ne // (CHUNK * P)

    nc = bacc.Bacc("TRN2", target_bir_lowering=False, debug=False,
                   num_devices=NCORE, num_swdge_queues=4)

    xT = nc.dram_tensor("xT", [P, N], f32, kind="ExternalInput")
    w_ext = nc.dram_tensor("w_ext", [P, HO + 2 * H], f32, kind="ExternalInput")
    c_rep = nc.dram_tensor("c_rep", [P, CHUNK * H], f32, kind="ExternalInput")
    bias_bc = nc.dram_tensor("bias_bc", [P, O], f32, kind="ExternalInput")
    attr_s = nc.dram_tensor("attr_s", [P, nblk], f32, kind="ExternalInput")
    reld_s = nc.dram_tensor("reld_s", [P, nblk], f32, kind="ExternalInput")
    idxA = nc.dram_tensor("idxA", [P, ne // 16], i16, kind="ExternalInput")
    idxB = nc.dram_tensor("idxB", [P, ne // 16], i16, kind="ExternalInput")
    y = nc.dram_tensor("y", [NPAD, O], f32, kind="ExternalOutput")

    tableA = nc.dram_tensor("tableA", [NROWT, ROW_A], f32, kind="Internal")
    tableB = nc.dram_tensor("tableB", [NROWT, ROW_B], f32, kind="Internal")

    with TileContext(nc) as tc:
        with (
            tc.tile_pool(name="persist", bufs=1) as pp,
            tc.tile_pool(name="h_sb", bufs=3) as hp,
            tc.tile_pool(name="ga", bufs=4) as gap,
            tc.tile_pool(name="gb", bufs=4) as gbp,
            tc.tile_pool(name="work", bufs=3) as wp,
            tc.tile_pool(name="blk", bufs=4) as bp,
            tc.tile_pool(name="psum_h", bufs=2, space="PSUM") as ps,
            tc.tile_pool(name="psum_num", bufs=3, space="PSUM") as psn,
            tc.tile_pool(name="psum_den", bufs=3, space="PSUM") as psd,
        ):
            nc.gpsimd.load_library(mlp)

            # ---- persistent loads ----
            xT_sb = pp.tile([P, N], f32)
            nc.sync.dma_start(out=xT_sb[:], in_=xT[:])
            wext_sb = pp.tile([P, HO + 2 * H], f32)
            nc.sync.dma_start(out=wext_sb[:], in_=w_ext[:])
            crep_sb = pp.tile([P, CHUNK * H], f32)
            nc.sync.dma_start(out=crep_sb[:], in_=c_rep[:])
            bias_sb = pp.tile([P, O], f32)
            nc.sync.dma_start(out=bias_sb[:], in_=bias_bc[:])
            attr_sb = pp.tile([P, nblk], f32)
            nc.sync.dma_start(out=attr_sb[:], in_=attr_s[:])
            reld_sb = pp.tile([P, nblk], f32)
            nc.sync.dma_start(out=reld_sb[:], in_=reld_s[:])
            idxA_sb = pp.tile([P, ne // 16], i16)
            nc.sync.dma_start(out=idxA_sb[:], in_=idxA[:])
            idxB_sb = pp.tile([P, ne // 16], i16)
            nc.sync.dma_start(out=idxB_sb[:], in_=idxB[:])

            iota_i = pp.tile([P, P], i32)
            nc.gpsimd.iota(iota_i[:], pattern=[[1, P]], base=0,
                           channel_multiplier=0)
            iota_f = pp.tile([P, P], f32)
            nc.vector.tensor_copy(iota_f[:], iota_i[:])

            asd_sb = pp.tile([P, N_NT, H], f32)     # a_dst staging
            out_sb = pp.tile([P, NT, O], f32)       # final output staging

            # ---- phase 1: h = xT.T @ W_ext, write tables ----
            for t in range(N_NT):
                m = min(P, N - t * P)
                hps = ps.tile([P, HO + 2 * H], f32, space="PSUM", tag="hps")
                nc.tensor.matmul(hps[:m, :], lhsT=xT_sb[:, t * P:t * P + m],
                                 rhs=wext_sb[:], start=True, stop=True)
                hsb = hp.tile([P, HO + H], f32, tag="hsb")
                # h | a_src  (scalar engine copy keeps DVE free)
                nc.scalar.copy(hsb[:m, :], hps[:m, 0:HO + H])
                # a_dst -> staging
                nc.vector.tensor_copy(asd_sb[:m, t, :], hps[:m, HO + H:])
                nc.sync.dma_start(out=tableA[t * P:t * P + m, 0:HO + H],
                                  in_=hsb[:m, :])
            # tableB write: row (t*128+p) cols 0:4 = asd_sb[p, t, :]
            nc.sync.dma_start(
                out=tableB.ap()[0:N_NT * P, 0:H].rearrange(
                    "(t p) h -> p t h", p=P),
                in_=asd_sb[:, 0:N_NT, :])

            # ---- phase 2: edge chunks ----
            for ch in range(nch):
                ga = gap.tile([P, CHUNK, ROW_A], f32, tag="ga")
                nc.gpsimd.dma_gather(ga[:], tableA.ap()[:, :],
                                     idxA_sb[:, ch * 64:(ch + 1) * 64],
                                     CHUNK * P, CHUNK * P, ROW_A,
                                     queue_num=ch % 3)
                gb = gbp.tile([P, CHUNK, ROW_B], f32, tag="gb")
                nc.gpsimd.dma_gather(gb[:], tableB.ap()[:, :],
                                     idxB_sb[:, ch * 64:(ch + 1) * 64],
                                     CHUNK * P, CHUNK * P, ROW_B,
                                     queue_num=3)

                CW = CHUNK * H  # 32
                al = wp.tile([P, CW], f32, tag="al")
                al3 = al[:].rearrange("p (b h) -> p b h", b=CHUNK)
                # alpha = a_src + a_dst
                nc.vector.tensor_tensor(al3, ga[:, :, HO:HO + H],
                                        gb[:, :, 0:H], Alu.add)
                # + attr*c
                ae = wp.tile([P, CW], f32, tag="ae")
                nc.vector.tensor_tensor(
                    ae[:].rearrange("p (b h) -> p b h", b=CHUNK),
                    attr_sb[:, ch * CHUNK:(ch + 1) * CHUNK]
                    .to_broadcast([P, CHUNK, H]),
                    crep_sb[:].rearrange("p (b h) -> p b h", b=CHUNK),
                    Alu.mult)
                nc.vector.tensor_tensor(al[:], al[:], ae[:], Alu.add)
                # leaky relu: max(x, 0.2x)
                lr = wp.tile([P, CW], f32, tag="lr")
                nc.vector.tensor_scalar(lr[:], al[:], NEG_SLOPE, None, Alu.mult)
                nc.vector.tensor_tensor(lr[:], lr[:], al[:], Alu.max)
                # p = exp
                pexp = wp.tile([P, CW], f32, tag="pexp")
                nc.scalar.activation(pexp[:], lr[:], Act.Exp)

                for b in range(CHUNK):
                    blk = ch * CHUNK + b
                    t = blk_tile[blk]
                    ind = bp.tile([P, P], f32, tag="ind")
                    nc.vector.tensor_scalar(
                        ind[:], iota_f[:], reld_sb[:, blk:blk + 1], None,
                        Alu.is_equal)
                    phg = bp.tile([P, HO], f32, tag="phg")
                    nc.vector.tensor_tensor(
                        phg[:].rearrange("p (h o) -> p h o", h=H),
                        ga[:, b, 0:HO].rearrange("p (h o) -> p h o", h=H),
                        pexp[:, b * H:(b + 1) * H].to_broadcast([P, H, O]),
                        Alu.mult)
                    if blk_first[blk]:
                        accn = psn.tile([P, HO], f32, space="PSUM", tag="an")
                        accd = psd.tile([P, H], f32, space="PSUM", tag="ad")
                        meta["psum_tiles"][t] = (accn, accd)
                    accn, accd = meta["psum_tiles"][t]
                    nc.tensor.matmul(accn[:], lhsT=ind[:], rhs=phg[:],
                                     start=blk_first[blk], stop=blk_last[blk],
                                     skip_group_check=True)
                    nc.tensor.matmul(accd[:], lhsT=ind[:],
                                     rhs=pexp[:, b * H:(b + 1) * H],
                                     start=blk_first[blk], stop=blk_last[blk],
                                     skip_group_check=True)
                    if blk_last[blk]:
                        # epilogue for tile t
                        den = bp.tile([P, H], f32, tag="den")
                        nc.vector.tensor_scalar(den[:], accd[:], 1e-16,
                                                None, Alu.max)
                        rec = bp.tile([P, H], f32, tag="rec")
                        nc.vector.reciprocal(rec[:], den[:])
                        onum = bp.tile([P, HO], f32, tag="onum")
                        nc.vector.tensor_tensor(
                            onum[:].rearrange("p (h o) -> p h o", h=H),
                            accn[:].rearrange("p (h o) -> p h o", h=H),
                            rec[:].to_broadcast([P, H, O]), Alu.mult)
                        hsum = bp.tile([P, O], f32, tag="hsum")
                        nc.vector.tensor_reduce(
                            hsum[:], onum[:].rearrange("p (h o) -> p o h", h=H),
                            axis=mybir.AxisListType.X, op=Alu.add)
                        nc.vector.tensor_scalar(hsum[:], hsum[:], 1.0 / H,
                                                None, Alu.mult)
                        nc.vector.tensor_tensor(out_sb[:, t, :], hsum[:],
                                                bias_sb[:], Alu.add)

            # ---- final output DMA: y[t*128+p, :] = out_sb[p, t, :] ----
            nc.sync.dma_start(
                out=y.ap().rearrange("(t p) o -> p t o", p=P),
                in_=out_sb[:])

    nc.compile()
    return nc


def _preprocess(inputs):
    x = np.asarray(inputs["x"], np.float32)
    edge_index = np.asarray(inputs["edge_index"])
    edge_attr = np.asarray(inputs["edge_attr"], np.float32)
    W_src = np.asarray(inputs["W_src"], np.float32)
    att_src = np.asarray(inputs["att_src"], np.float32)
    att_dst = np.asarray(inputs["att_dst"], np.float32)
    W_edge = np.asarray(inputs["W_edge"], np.float32)
    att_edge = np.asarray(inputs["att_edge"], np.float32)
    bias = np.asarray(inputs["bias"], np.float32)

    src = edge_index[0].astype(np.int64)
    dst = edge_index[1].astype(np.int64)

    # weight folds (host: weights-only preprocessing)
    W_flat = W_src.reshape(D, HO)                              # [128, 256]
    Wa_src = np.einsum("dho,ho->dh", W_src, att_src)           # [128, 4]
    Wa_dst = np.einsum("dho,ho->dh", W_src, att_dst)           # [128, 4]
    w_ext = np.concatenate([W_flat, Wa_src, Wa_dst], axis=1)   # [128, 264]
    c = np.einsum("ho,ho->h", W_edge, att_edge)                # [4]
    c_rep = np.tile(np.tile(c, CHUNK)[None, :], (P, 1)).astype(np.float32)
    bias_bc = np.tile(bias[None, :], (P, 1)).astype(np.float32)

    # per-(core) edge partition: half = dst // NHALF
    per_core = []
    for half in range(2):
        sel = np.nonzero((dst >= half * NHALF) & (dst < (half + 1) * NHALF))[0]
        ld = dst[sel] - half * NHALF
        order = np.argsort(ld, kind="stable")
        sel = sel[order]
        ld = ld[order]
        tiles = ld // P
        per_core.append((sel, ld, tiles))

    # shared block counts per tile: max over halves (same for all batches)
    cnt = np.zeros((2, NT), np.int64)
    for half in range(2):
        _, _, tiles = per_core[half]
        cnt[half] = np.bincount(tiles, minlength=NT)
    bt = np.maximum(1, -(-cnt.max(axis=0) // P))               # blocks per tile
    total = int(bt.sum())
    pad_to = -(-total // CHUNK) * CHUNK
    bt[NT - 1] += pad_to - total
    nblk = int(bt.sum())
    ne = nblk * P

    blk_tile = np.repeat(np.arange(NT), bt)
    starts = np.concatenate([[0], np.cumsum(bt)])
    blk_first = np.zeros(nblk, bool)
    blk_last = np.zeros(nblk, bool)
    blk_first[starts[:-1]] = True
    blk_last[starts[1:] - 1] = True

    meta = {"nblk": nblk, "blk_tile": blk_tile.tolist(),
            "blk_first": blk_first.tolist(), "blk_last": blk_last.tolist(),
            "psum_tiles": {}}

    # per-half slot arrays (shared by both batches of the half)
    half_arrays = []
    for half in range(2):
        sel, ld, tiles = per_core[half]
        srcg = np.zeros(ne, np.int64)          # pad -> row 0
        dstg = np.zeros(ne, np.int64)
        attr = np.zeros(ne, np.float32)
        reld = np.full(ne, -1.0, np.float32)   # pad -> no indicator match
        tcnt = np.bincount(tiles, minlength=NT)
        # slot index for each sorted edge: tile-major blocks
        ofs_in_tile = np.arange(len(sel)) - np.repeat(
            np.concatenate([[0], np.cumsum(tcnt)])[:-1], tcnt)
        slot = starts[tiles] * P + ofs_in_tile
        srcg[slot] = src[sel]
        dstg[slot] = dst[sel]
        attr[slot] = edge_attr[sel]
        reld[slot] = (ld - tiles * P).astype(np.float32)

        def wrap16(a):
            w = a.astype(np.int16).reshape(-1, 64, 16).transpose(2, 0, 1)
            return np.tile(w.reshape(16, -1), (8, 1)).copy()

        half_arrays.append({
            "idxA": wrap16(srcg),
            "idxB": wrap16(dstg),
            "attr_s": np.ascontiguousarray(attr.reshape(nblk, P).T),
            "reld_s": np.ascontiguousarray(reld.reshape(nblk, P).T),
        })

    in_maps = []
    for core in range(NCORE):
        b, half = core // 2, core % 2
        m = dict(half_arrays[half])
        m["xT"] = np.ascontiguousarray(x[b].T)
        m["w_ext"] = np.ascontiguousarray(w_ext)
        m["c_rep"] = c_rep
        m["bias_bc"] = bias_bc
        in_maps.append(m)
    return meta, in_maps


def kernel(**inputs):
    from concourse.bass_utils import run_bass_kernel_spmd

    meta, in_maps = _preprocess(inputs)
    key = meta["nblk"]
    if key not in _cache:
        _cache[key] = _build_program(meta)
    nc = _cache[key]

    res = run_bass_kernel_spmd(nc, in_maps, core_ids=list(range(NCORE)))
    out = np.empty((B, N, O), np.float32)
    for core in range(NCORE):
        b, half = core // 2, core % 2
        out[b, half * NHALF:(half + 1) * NHALF, :] = \
            res.results[core]["y"][:NHALF]
    return out


# revision 15
# speedup vs baseline: 1.7121x; 1.7121x over previous
"""GAT layer (PyG GATConv, concat=False, edge_dim=1) on 8 Trainium2 cores.

Sharding: core c owns destination nodes [1280c, 1280(c+1)) (last core 1040),
for ALL 4 batches. The graph is batch-independent, so the per-edge gather row
carries all 4 batches' source features at once, and the edge bookkeeping
(indicator matrices, descriptors) is shared across batches -- 4x less
descriptor-generation and indicator work than a (batch x range) split.

Per core:
  phase 1: h[b] = x[b] @ [W | Wa_src | Wa_dst] for all N nodes, 4 batches.
    tableA row (node n, bf16, 2304 B): 4 x [h_b (256 bf16) | a_src_b (4 f32,
    stored as 8 bf16 slots via bitcast) | pad] -> gathered per edge by src.
    tableB row (node n, f32 64 els): a_dst for 4 batches (16 f32) -> gathered
    once per destination tile (not per edge).
  phase 2: edges sorted by dst, 128-edge blocks per 128-node dst tile
    (block counts shared across cores = max, so one SPMD program).
    Per chunk (8 blocks): dma_gather source rows; alpha = a_src + attr*c
    (+ a_dst via IndT matmul from SBUF); leakyrelu via max(x, 0.2x);
    p = exp on ScalarE, broadcast-expanded to [b,h,o].
    Per block: Ind[e,n] = (rel_dst[e]==n) via iota+is_equal; PSUM
    accumulation accn += Ind.T @ (p * h_src)  (numerator, 1024 cols) and
    accd += Ind.T @ p (softmax denominator, 16 cols).  No max-subtraction:
    |alpha| <= ~10 here so exp is safe in f32, softmax unchanged.
  epilogue per tile: divide, mean over heads, + bias.
"""

import numpy as np

B, N, E, D, H, O = 4, 10000, 160000, 128, 4, 64
NEG_SLOPE = 0.2
P = 128
HO = H * O                        # 256
NPC = 1280                        # dst nodes per core
NT = NPC // P                     # 10 dst tiles per core
N_NT = -(-N // P)                 # 79 node tiles for h build
NROWT = N_NT * P                  # 10112 table rows
RB = 272                          # bf16 elems per batch seg: 256 h + 8 + 8
ROW_A = 1152                      # bf16 elems per tableA row (2304 B)
ROW_B = 64                        # f32 els per tableB row (256 B)
FW = B * HO                       # 1024: phg width
BH = B * H                        # 16
CHUNK = 8                         # blocks per gather call (1024 edges max)
NCORE = 8
NQ = 1                            # SWDGE queues used (Tile sem lanes are
                                  # queue-agnostic; >1 risks lane/queue clash)

_cache = {}


def _build_program(meta):
    import concourse.bacc as bacc
    import concourse.mybir as mybir
    from concourse.tile import TileContext
    from concourse.library_config import mlp

    f32 = mybir.dt.float32
    bf16 = mybir.dt.bfloat16
    i16 = mybir.dt.int16
    i32 = mybir.dt.int32
    Alu = mybir.AluOpType
    Act = mybir.ActivationFunctionType

    nblk = meta["nblk"]
    blk_tile = meta["blk_tile"]
    blk_first = meta["blk_first"]
    blk_last = meta["blk_last"]
    ne = nblk * P
    nch = ne // (CHUNK * P)

    nc = bacc.Bacc("TRN2", target_bir_lowering=False, debug=False,
                   num_devices=NCORE, num_swdge_queues=4)

    xT = nc.dram_tensor("xT", [B, P, N], f32, kind="ExternalInput")
    w_ext = nc.dram_tensor("w_ext", [P, HO + 2 * H], f32, kind="ExternalInput")
    crep = nc.dram_tensor("crep", [P, P], f32, kind="ExternalInput")
    bias_bc = nc.dram_tensor("bias_bc", [P, B * O], f32, kind="ExternalInput")
    attr_s = nc.dram_tensor("attr_s", [P, nblk], f32, kind="ExternalInput")
    reld_s = nc.dram_tensor("reld_s", [P, nblk], f32, kind="ExternalInput")
    relrow = nc.dram_tensor("relrow", [nch, P, CHUNK * P], f32,
                            kind="ExternalInput")
    idxA = nc.dram_tensor("idxA", [P, ne // 16], i16, kind="ExternalInput")
    idxT = nc.dram_tensor("idxT", [P, NT * P // 16], i16,
                          kind="ExternalInput")   # own-range node ids, 2x640
    y = nc.dram_tensor("y", [NPC, B * O], f32, kind="ExternalOutput")

    tableA = nc.dram_tensor("tableA", [NROWT, ROW_A], bf16, kind="Internal")
    tableB = nc.dram_tensor("tableB", [NROWT, ROW_B], f32, kind="Internal")

    with TileContext(nc) as tc:
        with (
            tc.tile_pool(name="persist", bufs=1) as pp,
            tc.tile_pool(name="psum_num", bufs=2, space="PSUM") as psn,
            tc.tile_pool(name="psum_den", bufs=2, space="PSUM") as psd,
        ):
            nc.gpsimd.load_library(mlp)

            # persistent small tiles
            crep_sb = pp.tile([P, P], f32)
            nc.sync.dma_start(out=crep_sb[:], in_=crep[:])
            bias_sb = pp.tile([P, B * O], f32)
            nc.sync.dma_start(out=bias_sb[:], in_=bias_bc[:])
            attr_sb = pp.tile([P, nblk], f32)
            nc.sync.dma_start(out=attr_sb[:], in_=attr_s[:])
            reld_sb = pp.tile([P, nblk], f32)
            nc.sync.dma_start(out=reld_sb[:], in_=reld_s[:])
            idxA_sb = pp.tile([P, ne // 16], i16)
            nc.sync.dma_start(out=idxA_sb[:], in_=idxA[:])
            idxT_sb = pp.tile([P, NT * P // 16], i16)
            nc.sync.dma_start(out=idxT_sb[:], in_=idxT[:])

            iota_i = pp.tile([P, P], i32)
            nc.gpsimd.iota(iota_i[:], pattern=[[1, P]], base=0,
                           channel_multiplier=0)
            iota_row = pp.tile([P, P], f32)       # [p, j] = j
            nc.vector.tensor_copy(iota_row[:], iota_i[:])
            iota_ci = pp.tile([P, 1], i32)
            nc.gpsimd.iota(iota_ci[:], pattern=[[0, 1]], base=0,
                           channel_multiplier=1)
            iota_col = pp.tile([P, 1], f32)       # [p, 0] = p
            nc.vector.tensor_copy(iota_col[:], iota_ci[:])

            asd_all = pp.tile([P, N_NT, BH], f32)
            nc.gpsimd.memset(asd_all[:], 0.0)
            out_sb = pp.tile([P, NT, B * O], f32)

            # ---- phase 1 ----
            with (
                tc.tile_pool(name="p1x", bufs=2) as p1x,
                tc.tile_pool(name="p1h", bufs=1) as p1h,
                tc.tile_pool(name="psum_h", bufs=2, space="PSUM") as psh,
            ):
                wext_sb = p1h.tile([P, HO + 2 * H], f32, tag="wext")
                nc.sync.dma_start(out=wext_sb[:], in_=w_ext[:])
                for b in range(B):
                    xT_sb = p1x.tile([P, N], f32, tag="xt")
                    nc.sync.dma_start(out=xT_sb[:], in_=xT.ap()[b])
                    hstage = p1h.tile([P, N_NT, RB], bf16, tag="hs")
                    nc.gpsimd.memset(hstage[:], 0.0)
                    for t in range(N_NT):
                        m = min(P, N - t * P)
                        hps = psh.tile([P, HO + 2 * H], f32, space="PSUM",
                                       tag="hps")
                        nc.tensor.matmul(hps[:m, :],
                                         lhsT=xT_sb[:, t * P:t * P + m],
                                         rhs=wext_sb[:], start=True, stop=True)
                        # h -> bf16
                        nc.scalar.copy(hstage[:m, t, 0:HO], hps[:m, 0:HO])
                        # a_src f32 bits into bf16 row via bitcast
                        nc.vector.tensor_copy(
                            hstage[:, t, :].bitcast(f32)[:m, HO // 2:HO // 2 + H],
                            hps[:m, HO:HO + H])
                        # a_dst staging
                        nc.vector.tensor_copy(asd_all[:m, t, b * H:(b + 1) * H],
                                              hps[:m, HO + H:])
                    # one big write per batch: tableA[., RB*b : RB*b+RB]
                    nc.sync.dma_start(
                        out=tableA.ap()[:, RB * b:RB * (b + 1)].rearrange(
                            "(t p) c -> p t c", p=P),
                        in_=hstage[:])
                # tableB write (all batches at once)
                nc.sync.dma_start(
                    out=tableB.ap()[:, 0:BH].rearrange("(t p) c -> p t c", p=P),
                    in_=asd_all[:])

            # ---- phase 2 ----
            with (
                tc.tile_pool(name="ga", bufs=3) as gap,
                tc.tile_pool(name="rr", bufs=2) as rrp,
                tc.tile_pool(name="wk", bufs=3) as wp,
                tc.tile_pool(name="bk", bufs=4) as bp,
                tc.tile_pool(name="psum_t", bufs=2, space="PSUM") as pst,
            ):
                # SWDGE queue must equal (pool-DMA issue index) % 4 so Tile's
                # round-robin DMASW lanes (8) pair consistently with queues.
                qctr = [0]

                def nextq():
                    v = qctr[0] % NQ
                    qctr[0] += 1
                    return v

                # a_dst rows for own range: 2 gathers of 640 rows
                asd_own = pp.tile([P, NT, ROW_B], f32)
                for gi in range(2):
                    nc.gpsimd.dma_gather(
                        asd_own[:, gi * (NT // 2):(gi + 1) * (NT // 2), :],
                        tableB.ap()[:, :],
                        idxT_sb[:, gi * 40:(gi + 1) * 40],
                        NT * P // 2, NT * P // 2, ROW_B, queue_num=nextq())

                for ch in range(nch):
                    ga = gap.tile([P, CHUNK, ROW_A], bf16, tag="ga")
                    nc.gpsimd.dma_gather(ga[:], tableA.ap()[:, :],
                                         idxA_sb[:, ch * 64:(ch + 1) * 64],
                                         CHUNK * P, CHUNK * P, ROW_A,
                                         queue_num=nextq())
                    rr = rrp.tile([P, CHUNK * P], f32, tag="rr")
                    nc.sync.dma_start(out=rr[:], in_=relrow.ap()[ch])

                    gaf = ga[:].bitcast(f32)      # [P, CHUNK, 576]
                    CW = CHUNK * BH               # 128
                    # alpha = attr*c + a_src
                    alc = wp.tile([P, CW], f32, tag="alc")
                    al3 = alc[:].rearrange("p (k c) -> p k c", k=CHUNK)
                    al4 = alc[:].rearrange("p (k b h) -> p k b h", k=CHUNK, b=B)
                    nc.vector.tensor_tensor(
                        al3,
                        attr_sb[:, ch * CHUNK:(ch + 1) * CHUNK]
                        .to_broadcast([P, CHUNK, BH]),
                        crep_sb[:].rearrange("p (k c) -> p k c", k=CHUNK),
                        Alu.mult)
                    # a_src: f32 view, batch seg stride 136, offset 128
                    nc.vector.tensor_tensor(
                        al4, al4,
                        gaf[:, :, 0:B * (RB // 2)].rearrange(
                            "p k (b c) -> p k b c", b=B)[:, :, :, HO // 2:
                                                         HO // 2 + H],
                        Alu.add)

                    for b8 in range(CHUNK):
                        blk = ch * CHUNK + b8
                        t = blk_tile[blk]
                        # IndT[n, e] = (relrow[e] == n)
                        indT = bp.tile([P, P], f32, tag="indT")
                        nc.vector.tensor_scalar(
                            indT[:], rr[:, b8 * P:(b8 + 1) * P],
                            iota_col[:, 0:1], None, Alu.is_equal)
                        adst = psd.tile([P, BH], f32, space="PSUM", tag="adst")
                        nc.tensor.matmul(adst[:], lhsT=indT[:],
                                         rhs=asd_own[:, t, 0:BH],
                                         start=True, stop=True)
                        # alpha += a_dst
                        nc.vector.tensor_tensor(
                            alc[:, b8 * BH:(b8 + 1) * BH],
                            alc[:, b8 * BH:(b8 + 1) * BH], adst[:], Alu.add)

                    # leaky relu: max(x, 0.2x)
                    lr = wp.tile([P, CW], f32, tag="lr")
                    nc.vector.tensor_scalar(lr[:], alc[:], NEG_SLOPE, None,
                                            Alu.mult)
                    nc.vector.tensor_tensor(lr[:], lr[:], alc[:], Alu.max)

                    for b8 in range(CHUNK):
                        blk = ch * CHUNK + b8
                        t = blk_tile[blk]
                        # p expanded to [b, h, o] on ScalarE (bf16 out)
                        px = bp.tile([P, FW], bf16, tag="px")
                        nc.scalar.activation(
                            px[:].rearrange("p (c o) -> p c o", c=BH),
                            lr[:, b8 * BH:(b8 + 1) * BH]
                            .to_broadcast([P, BH, O]),
                            Act.Exp)
                        # Ind[e, n] (bf16 for the bf16 matmuls)
                        ind = bp.tile([P, P], bf16, tag="ind")
                        nc.vector.tensor_scalar(
                            ind[:], iota_row[:], reld_sb[:, blk:blk + 1],
                            None, Alu.is_equal)
                        # phg = p * h_src
                        phg = bp.tile([P, FW], bf16, tag="phg")
                        nc.vector.tensor_tensor(
                            phg[:].rearrange("p (b h o) -> p b h o", b=B, h=H),
                            ga[:, b8, 0:B * RB].rearrange(
                                "p (b c) -> p b c", b=B)[:, :, 0:HO]
                            .rearrange("p b (h o) -> p b h o", h=H),
                            px[:].rearrange("p (b h o) -> p b h o", b=B, h=H),
                            Alu.mult)
                        if blk_first[blk]:
                            accn = psn.tile([P, FW], f32, space="PSUM",
                                            tag="an")
                            accd = pst.tile([P, BH], f32, space="PSUM",
                                            tag="ad")
                            meta["psum_tiles"][t] = (accn, accd)
                        accn, accd = meta["psum_tiles"][t]
                        nc.tensor.matmul(accn[:, 0:FW // 2], lhsT=ind[:],
                                         rhs=phg[:, 0:FW // 2],
                                         start=blk_first[blk],
                                         stop=blk_last[blk],
                                         skip_group_check=True)
                        nc.tensor.matmul(accn[:, FW // 2:], lhsT=ind[:],
                                         rhs=phg[:, FW // 2:],
                                         start=blk_first[blk],
                                         stop=blk_last[blk],
                                         skip_group_check=True)
                        nc.tensor.matmul(
                            accd[:], lhsT=ind[:],
                            rhs=px[:, 0:FW:O],
                            start=blk_first[blk], stop=blk_last[blk],
                            skip_group_check=True)

                        if blk_last[blk]:
                            den = bp.tile([P, BH], f32, tag="den")
                            nc.vector.tensor_scalar(den[:], accd[:], 1e-16,
                                                    None, Alu.max)
                            rec = bp.tile([P, BH], f32, tag="rec")
                            nc.vector.reciprocal(rec[:], den[:])
                            onum = bp.tile([P, FW], f32, tag="onum")
                            nc.vector.tensor_tensor(
                                onum[:].rearrange("p (c o) -> p c o", c=BH),
                                accn[:].rearrange("p (c o) -> p c o", c=BH),
                                rec[:].to_broadcast([P, BH, O]), Alu.mult)
                            hsum = bp.tile([P, B * O], f32, tag="hsum")
                            nc.vector.tensor_reduce(
                                hsum[:].rearrange("p (b o) -> p b o", b=B),
                                onum[:].rearrange("p (b h o) -> p b o h",
                                                  b=B, h=H),
                                axis=mybir.AxisListType.X, op=Alu.add)
                            nc.vector.tensor_scalar(hsum[:], hsum[:], 1.0 / H,
                                                    None, Alu.mult)
                            nc.vector.tensor_tensor(out_sb[:, t, :], hsum[:],
                                                    bias_sb[:], Alu.add)

                # final output
                nc.sync.dma_start(
                    out=y.ap().rearrange("(t p) o -> p t o", p=P),
                    in_=out_sb[:])

    nc.compile()
    return nc


def _preprocess(inputs):
    x = np.asarray(inputs["x"], np.float32)
    edge_index = np.asarray(inputs["edge_index"])
    edge_attr = np.asarray(inputs["edge_attr"], np.float32)
    W_src = np.asarray(inputs["W_src"], np.float32)
    att_src = np.asarray(inputs["att_src"], np.float32)
    att_dst = np.asarray(inputs["att_dst"], np.float32)
    W_edge = np.asarray(inputs["W_edge"], np.float32)
    att_edge = np.asarray(inputs["att_edge"], np.float32)
    bias = np.asarray(inputs["bias"], np.float32)

    src = edge_index[0].astype(np.int64)
    dst = edge_index[1].astype(np.int64)

    W_flat = W_src.reshape(D, HO)
    Wa_src = np.einsum("dho,ho->dh", W_src, att_src)
    Wa_dst = np.einsum("dho,ho->dh", W_src, att_dst)
    w_ext = np.ascontiguousarray(
        np.concatenate([W_flat, Wa_src, Wa_dst], axis=1))
    c = np.einsum("ho,ho->h", W_edge, att_edge)              # [4]
    # crep[p, 16k + 4b + h] = c[h]
    crep = np.tile(np.tile(c, B), CHUNK)[None, :].repeat(P, 0).copy()
    bias_bc = np.tile(bias, B)[None, :].repeat(P, 0).copy()

    # per-core dst ranges
    per_core = []
    cnt = np.zeros((NCORE, NT), np.int64)
    for core in range(NCORE):
        lo, hi = core * NPC, min((core + 1) * NPC, N)
        sel = np.nonzero((dst >= lo) & (dst < hi))[0]
        ld = dst[sel] - lo
        order = np.argsort(ld, kind="stable")
        sel, ld = sel[order], ld[order]
        tiles = ld // P
        cnt[core] = np.bincount(tiles, minlength=NT)
        per_core.append((sel, ld, tiles))

    bt = np.maximum(1, -(-cnt.max(axis=0) // P))
    total = int(bt.sum())
    bt[NT - 1] += -(-total // CHUNK) * CHUNK - total
    nblk = int(bt.sum())
    ne = nblk * P
    starts = np.concatenate([[0], np.cumsum(bt)])

    blk_tile = np.repeat(np.arange(NT), bt)
    blk_first = np.zeros(nblk, bool)
    blk_last = np.zeros(nblk, bool)
    blk_first[starts[:-1]] = True
    blk_last[starts[1:] - 1] = True

    meta = {"nblk": nblk, "blk_tile": blk_tile.tolist(),
            "blk_first": blk_first.tolist(), "blk_last": blk_last.tolist(),
            "psum_tiles": {}}

    def wrap16(a, chunklen=1024):
        # idx j of each chunklen-call -> partition j%16, col j//16; x8 replicate
        ncalls = len(a) // chunklen
        w = a.astype(np.int16).reshape(ncalls, chunklen // 16, 16)
        w = w.transpose(2, 0, 1).reshape(16, -1)
        return np.tile(w, (8, 1)).copy()

    in_maps = []
    for core in range(NCORE):
        sel, ld, tiles = per_core[core]
        srcg = np.zeros(ne, np.int64)
        attr = np.zeros(ne, np.float32)
        reld = np.full(ne, -1.0, np.float32)
        tcnt = np.bincount(tiles, minlength=NT)
        ofs = np.arange(len(sel)) - np.repeat(
            np.concatenate([[0], np.cumsum(tcnt)])[:-1], tcnt)
        slot = starts[tiles] * P + ofs
        srcg[slot] = src[sel]
        attr[slot] = edge_attr[sel]
        reld[slot] = (ld - tiles * P).astype(np.float32)

        own = (np.arange(NT * P) + core * NPC).clip(max=N - 1)
        nch = ne // 1024
        m = {
            "idxA": wrap16(srcg),
            "idxT": wrap16(own, chunklen=640),
            "attr_s": np.ascontiguousarray(attr.reshape(nblk, P).T),
            "reld_s": np.ascontiguousarray(reld.reshape(nblk, P).T),
            "relrow": np.ascontiguousarray(
                np.broadcast_to(reld.reshape(nch, 1, 1024), (nch, P, 1024))),
            "xT": np.ascontiguousarray(x.transpose(0, 2, 1)),
            "w_ext": w_ext, "crep": crep.astype(np.float32),
            "bias_bc": bias_bc.astype(np.float32),
        }
        in_maps.append(m)
    return meta, in_maps


def kernel(**inputs):
    from concourse.bass_utils import run_bass_kernel_spmd

    meta, in_maps = _preprocess(inputs)
    key = meta["nblk"]
    if key not in _cache:
        _cache[key] = _build_program(meta)
    nc = _cache[key]

    res = run_bass_kernel_spmd(nc, in_maps, core_ids=list(range(NCORE)))
    out = np.empty((B, N, O), np.float32)
    for core in range(NCORE):
        lo, hi = core * NPC, min((core + 1) * NPC, N)
        yc = res.results[core]["y"]                 # [1280, 256]
        for b in range(B):
            out[b, lo:hi, :] = yc[:hi - lo, b * O:(b + 1) * O]
    return out
